# revision 14
# baseline (speedup 1.0000x reference)
"""Bass/Trainium2 kernel: BidirectionalLSTMWithAttention, data-parallel x8.

See design notes at bottom of file. Self-contained: hardcodes all shapes.
"""
import contextlib
import numpy as np
import ml_dtypes

import concourse.bass as bass
import concourse.bacc as bacc
import concourse.mybir as mybir
from concourse.tile import TileContext
from concourse.bass_utils import run_bass_kernel_spmd
from concourse.alu_op_type import AluOpType

BF = ml_dtypes.bfloat16
F32 = mybir.dt.float32
BF16 = mybir.dt.bfloat16
AF = mybir.ActivationFunctionType
ALU = AluOpType

T = 512
BL = 64
NCORES = 8
DP = 16


# ----------------------------------------------------------------- host prep
def _prep_shared(params):
    p = {k: np.asarray(v, np.float32) for k, v in params.items()}
    rs = np.ones((512, 1), np.float32)
    rs[0:256] = 0.5    # i, f rows -> tanh(x/2) form
    rs[384:512] = 0.5  # o rows

    whh_img = np.zeros((128, 2048), np.float32)
    wih0_img = np.zeros((16, 1024), np.float32)
    wih1_img = np.zeros((128, 4096), np.float32)
    b0_img = np.zeros((8, 128), np.float32)
    b1_img = np.zeros((8, 128), np.float32)
    for l in (0, 1):
        for d, suf in enumerate(('', 'r')):
            whh = p[f'Whh_l{l}{suf}'] * rs * 0.5           # col x0.5: h'=2h
            wih = p[f'Wih_l{l}{suf}'] * rs
            if l == 1:
                wih = wih * 0.5
            bias = (p[f'bih_l{l}{suf}'] + p[f'bhh_l{l}{suf}']) * rs[:, 0]
            Loff = (l * 2 + d) * 512
            bimg = b0_img if l == 0 else b1_img
            for G in range(4):
                blk = whh[G * 128:(G + 1) * 128, :]
                whh_img[:, Loff + G * 128: Loff + (G + 1) * 128] = blk.T
                bimg[G * 2 + d, :] = bias[G * 128:(G + 1) * 128]
                if l == 0:
                    wb = wih[G * 128:(G + 1) * 128, :]
                    wih0_img[0:13, d * 512 + G * 128: d * 512 + (G + 1) * 128] = wb.T
                else:
                    for kc in (0, 1):
                        wb = wih[G * 128:(G + 1) * 128, kc * 128:(kc + 1) * 128]
                        off = ((kc * 2 + d) * 4 + G) * 128
                        wih1_img[:, off: off + 128] = wb.T

    bsel_img = np.zeros((8, 512), np.float32)
    for n in range(512):
        bsel_img[(n // 128) * 2 + (n // 64) % 2, n] = 1.0

    aw_img = np.zeros((128, 2), np.float32)
    aw = p['attn_w'][0] * 0.5
    aw_img[:, 0] = aw[0:128]
    aw_img[:, 1] = aw[128:256]

    mlpw = np.zeros((128, 326), np.float32)
    f1 = p['fc1_w'] * 0.5
    rw = p['res_w'] * 0.5
    mlpw[:, 0:64] = f1[:, 0:128].T
    mlpw[:, 64:128] = f1[:, 128:256].T
    mlpw[:, 128:192] = rw[:, 0:128].T
    mlpw[:, 192:256] = rw[:, 128:256].T
    mlpw[0:64, 256:320] = p['fc2_w'].T
    mlpw[0:64, 320:326] = p['out_w'].T
    mlpb = np.zeros((1, 198), np.float32)
    mlpb[0, 0:64] = p['fc1_b']
    mlpb[0, 64:128] = p['res_b']
    mlpb[0, 128:192] = p['fc2_b']
    mlpb[0, 192:198] = p['out_b']

    return {
        'whh': whh_img.astype(BF), 'wih0': wih0_img.astype(BF),
        'wih1': wih1_img.astype(BF), 'b0': b0_img.astype(BF),
        'b1': b1_img.astype(BF), 'bsel': bsel_img.astype(BF),
        'aw': aw_img.astype(BF), 'ident': np.eye(128, dtype=np.float32),
        'mlpw': mlpw, 'mlpb': mlpb,
    }


def _prep_x(x_core, Tn=T):
    xw = np.asarray(x_core, np.float32).reshape(BL, Tn, 13)
    xp = np.zeros((BL, Tn, DP), np.float32)
    xp[:, :, 0:13] = xw
    xt = np.transpose(xp, (2, 1, 0)).reshape(DP, Tn * BL)   # [d, t*64+b]
    return np.ascontiguousarray(xt).astype(BF)


# ------------------------------------------------------------------ program
def build(Tn=T):
    nc = bacc.Bacc('TRN2', target_bir_lowering=False, debug=False)
    NCH = Tn // 8

    xt_d = nc.dram_tensor('xt', [DP, Tn * BL], BF16, kind='ExternalInput')
    whh_d = nc.dram_tensor('whh', [128, 2048], BF16, kind='ExternalInput')
    wih0_d = nc.dram_tensor('wih0', [16, 1024], BF16, kind='ExternalInput')
    wih1_d = nc.dram_tensor('wih1', [128, 4096], BF16, kind='ExternalInput')
    b0_d = nc.dram_tensor('b0', [8, 128], BF16, kind='ExternalInput')
    b1_d = nc.dram_tensor('b1', [8, 128], BF16, kind='ExternalInput')
    bsel_d = nc.dram_tensor('bsel', [8, 512], BF16, kind='ExternalInput')
    aw_d = nc.dram_tensor('aw', [128, 2], BF16, kind='ExternalInput')
    ident_d = nc.dram_tensor('ident', [128, 128], F32, kind='ExternalInput')
    mlpw_d = nc.dram_tensor('mlpw', [128, 326], F32, kind='ExternalInput')
    mlpb_d = nc.dram_tensor('mlpb', [1, 198], F32, kind='ExternalInput')
    out_d = nc.dram_tensor('out', [BL, 6], F32, kind='ExternalOutput')
    h0_d = nc.dram_tensor('h0scratch', [2, Tn, 128, BL], BF16)

    ctx = contextlib.ExitStack()
    with TileContext(nc) as tc, ctx:
        const = ctx.enter_context(tc.tile_pool(name='const', bufs=1))
        whh = const.tile([128, 2048], BF16)
        wih0 = const.tile([16, 1024], BF16)
        wih1 = const.tile([128, 4096], BF16)
        b0 = const.tile([8, 128], BF16)
        b1 = const.tile([8, 128], BF16)
        bsel = const.tile([8, 512], BF16)
        aw = const.tile([128, 2], BF16)
        ident = const.tile([128, 128], F32)
        mlpw = const.tile([128, 326], F32)
        mlpb = const.tile([1, 198], F32)
        ones = const.tile([1, 64], F32)
        h1T = const.tile([128, Tn * 128], BF16)  # [(chain,b), (step,k)]

        for sb, dr in ((whh, whh_d), (wih0, wih0_d), (wih1, wih1_d),
                       (b0, b0_d), (b1, b1_d), (bsel, bsel_d), (aw, aw_d),
                       (ident, ident_d), (mlpw, mlpw_d), (mlpb, mlpb_d)):
            nc.sync.dma_start(sb[:], dr[:])
        nc.vector.memset(ones[:], 1.0)

        state = ctx.enter_context(tc.tile_pool(name='state', bufs=2))
        work = ctx.enter_context(tc.tile_pool(name='work', bufs=3))
        hin = ctx.enter_context(tc.tile_pool(name='hin', bufs=6))
        gpool = ctx.enter_context(tc.tile_pool(name='gates', bufs=3, space='PSUM'))
        spool = ctx.enter_context(tc.tile_pool(name='spsum', bufs=1, space='PSUM'))
        ppool = ctx.enter_context(tc.tile_pool(name='post', bufs=2, space='PSUM'))

        # scores psum: rows 0:64 = s[t,b] at col t; rows 64:128 = s[t,b] at
        # col Tn-1-t (pre-reversed copy for the bwd-chain context scan)
        S = spool.tile([128, Tn], F32)

        def lstm_phase(layer):
            bias = b0 if layer == 0 else b1
            Lw = layer * 1024
            h_prev = state.tile([128, 128], BF16, tag='h')
            s_prev = state.tile([128, 128], F32, tag='s')
            nc.vector.memset(h_prev[:], 0.0)
            nc.vector.memset(s_prev[:], 0.0)
            for step in range(Tn):
                tf, tb = step, Tn - 1 - step
                g = gpool.tile([128, 512], F32, tag='g')
                nc.tensor.matmul(g[:], bias[:], bsel[:], start=True, stop=False,
                                 skip_group_check=True)
                if layer == 0:
                    xA = hin.tile([16, 64], BF16, tag='xA')
                    xB = hin.tile([16, 64], BF16, tag='xB')
                    nc.sync.dma_start(xA[:], xt_d[:, tf * BL:(tf + 1) * BL])
                    nc.sync.dma_start(xB[:], xt_d[:, tb * BL:(tb + 1) * BL])
                if layer == 1:
                    hA = hin.tile([128, 128], BF16, tag='hA')
                    hB = hin.tile([128, 128], BF16, tag='hB')
                    nc.sync.dma_start(hA[:, 0:64], h0_d[0, tf])
                    nc.sync.dma_start(hA[:, 64:128], h0_d[1, tf])
                    nc.sync.dma_start(hB[:, 0:64], h0_d[0, tb])
                    nc.sync.dma_start(hB[:, 64:128], h0_d[1, tb])
                for d in (0, 1):
                    t = tf if d == 0 else tb
                    for G in range(4):
                        reg = g[:, G * 128 + d * 64: G * 128 + d * 64 + 64]
                        nc.tensor.matmul(
                            reg,
                            whh[:, Lw + d * 512 + G * 128: Lw + d * 512 + (G + 1) * 128],
                            h_prev[:, d * 64: d * 64 + 64],
                            start=False, stop=False, skip_group_check=True)
                        if layer == 0:
                            xX = xA if d == 0 else xB
                            nc.tensor.matmul(
                                reg,
                                wih0[0:16, d * 512 + G * 128: d * 512 + (G + 1) * 128],
                                xX[:],
                                start=False, stop=True, skip_group_check=True)
                        else:
                            hX = hA if d == 0 else hB
                            for kc in (0, 1):
                                off = ((kc * 2 + d) * 4 + G) * 128
                                nc.tensor.matmul(
                                    reg, wih1[:, off: off + 128],
                                    hX[:, kc * 64:(kc + 1) * 64],
                                    start=False, stop=(kc == 1),
                                    skip_group_check=True)
                tt = work.tile([128, 512], F32, tag='tt')
                nc.scalar.activation(tt[:], g[:], AF.Tanh)
                A = work.tile([128, 128], F32, tag='A')
                B2 = work.tile([128, 128], F32, tag='B')
                s_new = state.tile([128, 128], F32, tag='s')
                Tc = work.tile([128, 128], F32, tag='Tc')
                h_new = state.tile([128, 128], BF16, tag='h')
                nc.vector.scalar_tensor_tensor(
                    A[:], tt[:, 128:256], 1.0, s_prev[:], op0=ALU.add, op1=ALU.mult)
                nc.vector.scalar_tensor_tensor(
                    B2[:], tt[:, 0:128], 1.0, tt[:, 256:384], op0=ALU.add, op1=ALU.mult)
                nc.vector.scalar_tensor_tensor(
                    s_new[:], A[:], 0.5, B2[:], op0=ALU.mult, op1=ALU.add)
                nc.scalar.activation(Tc[:], s_new[:], AF.Tanh, scale=0.5)
                nc.vector.scalar_tensor_tensor(
                    h_new[:], tt[:, 384:512], 1.0, Tc[:], op0=ALU.add, op1=ALU.mult)
                if layer == 0:
                    nc.sync.dma_start(h0_d[0, tf], h_new[:, 0:64])
                    nc.sync.dma_start(h0_d[1, tb], h_new[:, 64:128])
                else:
                    first = step < Tn // 2
                    for d, t in ((0, tf), (1, tb)):
                        hs = h_new[:, d * 64:(d + 1) * 64]
                        nc.tensor.matmul(
                            S[0:64, t: t + 1], hs, aw[:, d:d + 1],
                            start=first, stop=not first, skip_group_check=True)
                        nc.tensor.matmul(
                            S[64:128, Tn - 1 - t: Tn - t], hs, aw[:, d:d + 1],
                            start=first, stop=not first, skip_group_check=True)
                    nc.sync.dma_start_transpose(
                        h1T[:, step * 128:(step + 1) * 128], h_new[:])
                h_prev, s_prev = h_new, s_new

        lstm_phase(0)
        lstm_phase(1)

        # ------------------------------------------------ attention + MLP
        Es = const.tile([128, Tn], F32)
        nc.scalar.activation(Es[:], S[:], AF.Exp)
        rec = const.tile([64, 1], F32)
        den = const.tile([64, 1], F32)
        nc.vector.reduce_sum(den[:], Es[0:64, :], axis=mybir.AxisListType.X)
        nc.vector.reciprocal(rec[:], den[:])
        rfull = const.tile([128, 1], F32)
        nc.vector.tensor_copy(rfull[0:64, :], rec[:])
        nc.vector.tensor_copy(rfull[64:128, :], rec[:])
        Wf = const.tile([128, Tn], F32)
        nc.vector.scalar_tensor_tensor(
            Wf[:], Es[:], rfull[:, 0:1], Es[:], op0=ALU.mult, op1=ALU.bypass)
        # context scan: acc[(chain,b), k] += h1T[:, s*128+k] * Wf[(chain,b), s]
        acc = const.tile([128, 128], F32)
        nc.vector.memset(acc[:], 0.0)
        for s in range(Tn):
            nc.vector.scalar_tensor_tensor(
                acc[:], h1T[:, s * 128:(s + 1) * 128], Wf[:, s:s + 1], acc[:],
                op0=ALU.mult, op1=ALU.add)
        cp1 = const.tile([64, 128], F32)
        nc.vector.tensor_copy(cp1[:], acc[64:128, :])

        ctx0 = const.tile([128, 64], F32)
        ctx1 = const.tile([128, 64], F32)
        for src, dst in ((acc[0:64, :], ctx0), (cp1[:], ctx1)):
            pt = ppool.tile([128, 64], F32, tag='pt')
            nc.tensor.transpose(pt[:], src, ident[0:64, 0:64])
            nc.scalar.copy(dst[:], pt[:])

        # fc1 / res:  psum [64b, 64fc]
        f1p = ppool.tile([64, 64], F32, tag='mlp')
        nc.tensor.matmul(f1p[:], ctx0[:], mlpw[:, 0:64], start=True, stop=False)
        nc.tensor.matmul(f1p[:], ctx1[:], mlpw[:, 64:128], start=False, stop=False)
        nc.tensor.matmul(f1p[:], ones[:], mlpb[0:1, 0:64], start=False, stop=True)
        r1 = const.tile([64, 64], F32)
        nc.scalar.activation(r1[:], f1p[:], AF.Relu)
        rsp = ppool.tile([64, 64], F32, tag='mlp')
        nc.tensor.matmul(rsp[:], ctx0[:], mlpw[:, 128:192], start=True, stop=False)
        nc.tensor.matmul(rsp[:], ctx1[:], mlpw[:, 192:256], start=False, stop=False)
        nc.tensor.matmul(rsp[:], ones[:], mlpb[0:1, 64:128], start=False, stop=True)
        u = const.tile([64, 64], F32)
        nc.vector.tensor_tensor(u[:], r1[:], rsp[:], op=ALU.add)
        uptp = ppool.tile([64, 64], F32, tag='mlp')
        nc.tensor.transpose(uptp[:], u[:], ident[0:64, 0:64])
        uT = const.tile([64, 64], F32)
        nc.scalar.copy(uT[:], uptp[:])
        f2p = ppool.tile([64, 64], F32, tag='mlp')
        nc.tensor.matmul(f2p[:], mlpw[0:64, 256:320], uT[:], start=True, stop=False)
        nc.tensor.matmul(f2p[:], mlpb[0:1, 128:192], ones[:], start=False, stop=True)
        r2 = const.tile([64, 64], F32)
        nc.scalar.activation(r2[:], f2p[:], AF.Relu)
        op = ppool.tile([64, 6], F32, tag='mlp')
        nc.tensor.matmul(op[:], r2[:], mlpw[0:64, 320:326], start=True, stop=False)
        nc.tensor.matmul(op[:], ones[:], mlpb[0:1, 192:198], start=False, stop=True)
        ofin = const.tile([64, 6], F32)
        nc.scalar.copy(ofin[:], op[:])
        nc.sync.dma_start(out_d[:], ofin[:])
    nc.compile()
    return nc


_CACHE = {}


def kernel(x, params):
    x = np.asarray(x, np.float32)
    shared = _prep_shared(params)
    if T not in _CACHE:
        _CACHE[T] = build(T)
    nc = _CACHE[T]
    in_maps = []
    for i in range(NCORES):
        m = dict(shared)
        m['xt'] = _prep_x(x[i * BL:(i + 1) * BL])
        in_maps.append(m)
    res = run_bass_kernel_spmd(nc, in_maps, list(range(NCORES)))
    outs = [np.asarray(res.results[i]['out'], np.float32) for i in range(NCORES)]
    return np.concatenate(outs, axis=0)


if __name__ == '__main__':
    build(64)
    print('build ok')


# revision 22
# speedup vs baseline: 1.2426x; 1.2426x over previous
"""Bass/Trainium2 kernel: BidirectionalLSTMWithAttention, data-parallel x8.

See design notes at bottom of file. Self-contained: hardcodes all shapes.
"""
import contextlib
import numpy as np
import ml_dtypes

import concourse.bass as bass
import concourse.bacc as bacc
import concourse.mybir as mybir
from concourse.tile import TileContext
from concourse.bass_utils import run_bass_kernel_spmd
from concourse.alu_op_type import AluOpType

BF = ml_dtypes.bfloat16
F32 = mybir.dt.float32
BF16 = mybir.dt.bfloat16
AF = mybir.ActivationFunctionType
ALU = AluOpType

T = 512
BL = 64
NCORES = 8
DP = 16


# ----------------------------------------------------------------- host prep
def _prep_shared(params):
    p = {k: np.asarray(v, np.float32) for k, v in params.items()}
    rs = np.ones((512, 1), np.float32)
    rs[0:256] = 0.5    # i, f rows -> tanh(x/2) form
    rs[384:512] = 0.5  # o rows

    whh_img = np.zeros((128, 2048), np.float32)
    wih0_img = np.zeros((16, 1024), np.float32)
    wih1_img = np.zeros((128, 4096), np.float32)
    b0_img = np.zeros((8, 128), np.float32)
    b1_img = np.zeros((8, 128), np.float32)
    for l in (0, 1):
        for d, suf in enumerate(('', 'r')):
            whh = p[f'Whh_l{l}{suf}'] * rs * 0.5           # col x0.5: h'=2h
            wih = p[f'Wih_l{l}{suf}'] * rs
            if l == 1:
                wih = wih * 0.5
            bias = (p[f'bih_l{l}{suf}'] + p[f'bhh_l{l}{suf}']) * rs[:, 0]
            Loff = (l * 2 + d) * 512
            bimg = b0_img if l == 0 else b1_img
            for G in range(4):
                blk = whh[G * 128:(G + 1) * 128, :]
                whh_img[:, Loff + G * 128: Loff + (G + 1) * 128] = blk.T
                bimg[G * 2 + d, :] = bias[G * 128:(G + 1) * 128]
                if l == 0:
                    wb = wih[G * 128:(G + 1) * 128, :]
                    wih0_img[0:13, d * 512 + G * 128: d * 512 + (G + 1) * 128] = wb.T
                else:
                    for kc in (0, 1):
                        wb = wih[G * 128:(G + 1) * 128, kc * 128:(kc + 1) * 128]
                        off = ((kc * 2 + d) * 4 + G) * 128
                        wih1_img[:, off: off + 128] = wb.T

    bsel_img = np.zeros((8, 512), np.float32)
    for n in range(512):
        bsel_img[(n // 128) * 2 + (n // 64) % 2, n] = 1.0

    aw_img = np.zeros((128, 2), np.float32)
    aw = p['attn_w'][0] * 0.5
    aw_img[:, 0] = aw[0:128]
    aw_img[:, 1] = aw[128:256]

    mlpw = np.zeros((128, 326), np.float32)
    f1 = p['fc1_w'] * 0.5
    rw = p['res_w'] * 0.5
    mlpw[:, 0:64] = f1[:, 0:128].T
    mlpw[:, 64:128] = f1[:, 128:256].T
    mlpw[:, 128:192] = rw[:, 0:128].T
    mlpw[:, 192:256] = rw[:, 128:256].T
    mlpw[0:64, 256:320] = p['fc2_w'].T
    mlpw[0:64, 320:326] = p['out_w'].T
    mlpb = np.zeros((1, 198), np.float32)
    mlpb[0, 0:64] = p['fc1_b']
    mlpb[0, 64:128] = p['res_b']
    mlpb[0, 128:192] = p['fc2_b']
    mlpb[0, 192:198] = p['out_b']

    return {
        'whh': whh_img.astype(BF), 'wih0': wih0_img.astype(BF),
        'wih1': wih1_img.astype(BF), 'b0': b0_img.astype(BF),
        'b1': b1_img.astype(BF), 'bsel': bsel_img.astype(BF),
        'aw': aw_img.astype(BF), 'ident': np.eye(128, dtype=np.float32),
        'mlpw': mlpw, 'mlpb': mlpb,
    }


def _prep_x(x_core, Tn=T):
    xw = np.asarray(x_core, np.float32).reshape(BL, Tn, 13)
    xp = np.zeros((BL, Tn, DP), np.float32)
    xp[:, :, 0:13] = xw
    xt = np.transpose(xp, (2, 1, 0)).reshape(DP, Tn * BL)   # [d, t*64+b]
    return np.ascontiguousarray(xt).astype(BF)


# ------------------------------------------------------------------ program
def build(Tn=T):
    nc = bacc.Bacc('TRN2', target_bir_lowering=False, debug=False)
    NCH = Tn // 8

    xt_d = nc.dram_tensor('xt', [DP, Tn * BL], BF16, kind='ExternalInput')
    whh_d = nc.dram_tensor('whh', [128, 2048], BF16, kind='ExternalInput')
    wih0_d = nc.dram_tensor('wih0', [16, 1024], BF16, kind='ExternalInput')
    wih1_d = nc.dram_tensor('wih1', [128, 4096], BF16, kind='ExternalInput')
    b0_d = nc.dram_tensor('b0', [8, 128], BF16, kind='ExternalInput')
    b1_d = nc.dram_tensor('b1', [8, 128], BF16, kind='ExternalInput')
    bsel_d = nc.dram_tensor('bsel', [8, 512], BF16, kind='ExternalInput')
    aw_d = nc.dram_tensor('aw', [128, 2], BF16, kind='ExternalInput')
    ident_d = nc.dram_tensor('ident', [128, 128], F32, kind='ExternalInput')
    mlpw_d = nc.dram_tensor('mlpw', [128, 326], F32, kind='ExternalInput')
    mlpb_d = nc.dram_tensor('mlpb', [1, 198], F32, kind='ExternalInput')
    out_d = nc.dram_tensor('out', [BL, 6], F32, kind='ExternalOutput')
    # h0_d[s] = phase-1 step-s h' tile: cols 0:64 fwd pos s, 64:128 bwd pos Tn-1-s
    h0_d = nc.dram_tensor('h0scratch', [Tn, 128, 128], BF16)

    ctx = contextlib.ExitStack()
    with TileContext(nc) as tc, ctx:
        const = ctx.enter_context(tc.tile_pool(name='const', bufs=1))
        whh = const.tile([128, 2048], BF16)
        wih0 = const.tile([16, 1024], BF16)
        wih1 = const.tile([128, 4096], BF16)
        b0 = const.tile([8, 128], BF16)
        b1 = const.tile([8, 128], BF16)
        bsel = const.tile([8, 512], BF16)
        aw = const.tile([128, 2], BF16)
        ident = const.tile([128, 128], F32)
        mlpw = const.tile([128, 326], F32)
        mlpb = const.tile([1, 198], F32)
        ones = const.tile([1, 64], F32)
        h1T = const.tile([128, Tn * 128], BF16)  # [(chain,b), (step,k)]

        for sb, dr in ((whh, whh_d), (wih0, wih0_d), (wih1, wih1_d),
                       (b0, b0_d), (b1, b1_d), (bsel, bsel_d), (aw, aw_d),
                       (ident, ident_d), (mlpw, mlpw_d), (mlpb, mlpb_d)):
            nc.sync.dma_start(sb[:], dr[:])
        nc.vector.memset(ones[:], 1.0)

        state = ctx.enter_context(tc.tile_pool(name='state', bufs=2))
        stage = ctx.enter_context(tc.tile_pool(name='stage', bufs=3))
        work = ctx.enter_context(tc.tile_pool(name='work', bufs=3))
        hin = ctx.enter_context(tc.tile_pool(name='hin', bufs=3))
        gpool = ctx.enter_context(tc.tile_pool(name='gates', bufs=3, space='PSUM'))
        spool = ctx.enter_context(tc.tile_pool(name='spsum', bufs=1, space='PSUM'))
        ppool = ctx.enter_context(tc.tile_pool(name='post', bufs=2, space='PSUM'))

        # scores psum: rows 0:64 = s[t,b] at col t; rows 64:128 = s[t,b] at
        # col Tn-1-t (pre-reversed copy for the bwd-chain context scan)
        S = spool.tile([128, Tn], F32)

        h1T3 = h1T[:].rearrange('p (t k) -> p t k', k=128)

        def lstm_phase(layer):
            bias = b0 if layer == 0 else b1
            Lw = layer * 1024
            hz = state.tile([128, 128], BF16, tag='hz')
            s_prev = state.tile([128, 128], F32, tag='s')
            nc.vector.memset(hz[:], 0.0)
            nc.vector.memset(s_prev[:], 0.0)
            h_prev = hz[:]
            stg_prev = None
            xAb = xBb = hAb = hBb = None
            pend_scores = []

            def emit_scores():
                while pend_scores:
                    hN, ptf, ptb, first = pend_scores.pop()
                    for d, t in ((0, ptf), (1, ptb)):
                        hs = hN[:, d * 64:(d + 1) * 64]
                        nc.tensor.matmul(
                            S[0:64, t: t + 1], hs, aw[:, d:d + 1],
                            start=first, stop=not first, skip_group_check=True)
                        nc.tensor.matmul(
                            S[64:128, Tn - 1 - t: Tn - t], hs, aw[:, d:d + 1],
                            start=first, stop=not first, skip_group_check=True)
            for grp in range(Tn // 4):
                s0 = grp * 4
                tb0 = Tn - 1 - s0
                stg = stage.tile([128, 512], BF16, tag='stg')
                if layer == 0 and grp % 2 == 0:
                    xAb = hin.tile([16, 512], BF16, tag='xA')
                    xBb = hin.tile([16, 512], BF16, tag='xB')
                    nc.gpsimd.dma_start(xAb[:], xt_d[:, s0 * BL:(s0 + 8) * BL])
                    nc.gpsimd.dma_start(xBb[:], xt_d[:, (tb0 - 7) * BL:(tb0 + 1) * BL])
                if layer == 1:
                    hAb = hin.tile([128, 512], BF16, tag='hA')
                    hBb = hin.tile([128, 512], BF16, tag='hB')
                    nc.gpsimd.dma_start(
                        hAb[:].rearrange('p (a c) -> p a c', a=4),
                        h0_d[s0:s0 + 4].rearrange('a p c -> p a c'))
                    nc.gpsimd.dma_start(
                        hBb[:].rearrange('p (a c) -> p a c', a=4),
                        h0_d[tb0 - 3:tb0 + 1].rearrange('a p c -> p a c'))
                for j in range(4):
                    step = s0 + j
                    tf, tb = step, Tn - 1 - step
                    g = gpool.tile([128, 512], F32, tag='g')
                    nc.tensor.matmul(g[:], bias[:], bsel[:], start=True, stop=False,
                                     skip_group_check=True)
                    for d in (0, 1):
                        for G in range(4):
                            reg = g[:, G * 128 + d * 64: G * 128 + d * 64 + 64]
                            nc.tensor.matmul(
                                reg,
                                whh[:, Lw + d * 512 + G * 128: Lw + d * 512 + (G + 1) * 128],
                                h_prev[:, d * 64: d * 64 + 64],
                                start=False, stop=False, skip_group_check=True)
                            if layer == 0:
                                xs = (xAb[:, (step % 8) * 64:(step % 8) * 64 + 64]
                                      if d == 0 else
                                      xBb[:, (7 - step % 8) * 64:(8 - step % 8) * 64])
                                nc.tensor.matmul(
                                    reg,
                                    wih0[0:16, d * 512 + G * 128: d * 512 + (G + 1) * 128],
                                    xs, start=False, stop=True, skip_group_check=True)
                            else:
                                # input halves: kc0 = h_f(t), kc1 = h_b(t)
                                if d == 0:
                                    rhss = (hAb[:, j * 128: j * 128 + 64],
                                            hBb[:, (3 - j) * 128 + 64:(3 - j) * 128 + 128])
                                else:
                                    rhss = (hBb[:, (3 - j) * 128:(3 - j) * 128 + 64],
                                            hAb[:, j * 128 + 64: j * 128 + 128])
                                for kc in (0, 1):
                                    off = ((kc * 2 + d) * 4 + G) * 128
                                    nc.tensor.matmul(
                                        reg, wih1[:, off: off + 128], rhss[kc],
                                        start=False, stop=(kc == 1),
                                        skip_group_check=True)
                    emit_scores()
                    tt = work.tile([128, 512], F32, tag='tt')
                    nc.scalar.activation(tt[:], g[:], AF.Tanh)
                    A = work.tile([128, 128], F32, tag='A')
                    B2 = work.tile([128, 128], F32, tag='B')
                    s_new = state.tile([128, 128], F32, tag='s')
                    Tc = work.tile([128, 128], F32, tag='Tc')
                    h_new = stg[:, j * 128:(j + 1) * 128]
                    nc.gpsimd.scalar_tensor_tensor(
                        B2[:], tt[:, 0:128], 1.0, tt[:, 256:384], op0=ALU.add, op1=ALU.mult)
                    nc.vector.scalar_tensor_tensor(
                        A[:], tt[:, 128:256], 1.0, s_prev[:], op0=ALU.add, op1=ALU.mult)
                    nc.vector.scalar_tensor_tensor(
                        s_new[:], A[:], 0.5, B2[:], op0=ALU.mult, op1=ALU.add)
                    nc.scalar.activation(Tc[:], s_new[:], AF.Tanh, scale=0.5)
                    nc.vector.scalar_tensor_tensor(
                        h_new, tt[:, 384:512], 1.0, Tc[:], op0=ALU.add, op1=ALU.mult)
                    if layer == 1:
                        pend_scores.append((h_new, tf, tb, step < Tn // 2))
                    h_prev, s_prev = h_new, s_new
                if layer == 0:
                    nc.sync.dma_start(
                        h0_d[s0:s0 + 4].rearrange('a p c -> p a c'),
                        stg[:].rearrange('p (a c) -> p a c', a=4))
                else:
                    nc.sync.dma_start_transpose(h1T3[:, s0:s0 + 4, :], stg[:])
                stg_prev = stg
            emit_scores()

        lstm_phase(0)
        lstm_phase(1)

        # ------------------------------------------------ attention + MLP
        Es = const.tile([128, Tn], F32)
        nc.scalar.activation(Es[:], S[:], AF.Exp)
        rec = const.tile([64, 1], F32)
        den = const.tile([64, 1], F32)
        nc.vector.reduce_sum(den[:], Es[0:64, :], axis=mybir.AxisListType.X)
        nc.vector.reciprocal(rec[:], den[:])
        rfull = const.tile([128, 1], F32)
        nc.vector.tensor_copy(rfull[0:64, :], rec[:])
        nc.vector.tensor_copy(rfull[64:128, :], rec[:])
        Wf = const.tile([128, Tn], F32)
        nc.vector.scalar_tensor_tensor(
            Wf[:], Es[:], rfull[:, 0:1], Es[:], op0=ALU.mult, op1=ALU.bypass)
        # context scan: acc[(chain,b), k] += h1T[:, s*128+k] * Wf[(chain,b), s]
        acc = const.tile([128, 128], F32)
        nc.vector.memset(acc[:], 0.0)
        for s in range(Tn):
            nc.vector.scalar_tensor_tensor(
                acc[:], h1T[:, s * 128:(s + 1) * 128], Wf[:, s:s + 1], acc[:],
                op0=ALU.mult, op1=ALU.add)
        cp1 = const.tile([64, 128], F32)
        nc.vector.tensor_copy(cp1[:], acc[64:128, :])

        ctx0 = const.tile([128, 64], F32)
        ctx1 = const.tile([128, 64], F32)
        for src, dst in ((acc[0:64, :], ctx0), (cp1[:], ctx1)):
            pt = ppool.tile([128, 64], F32, tag='pt')
            nc.tensor.transpose(pt[:], src, ident[0:64, 0:64])
            nc.scalar.copy(dst[:], pt[:])

        # fc1 / res:  psum [64b, 64fc]
        f1p = ppool.tile([64, 64], F32, tag='mlp')
        nc.tensor.matmul(f1p[:], ctx0[:], mlpw[:, 0:64], start=True, stop=False)
        nc.tensor.matmul(f1p[:], ctx1[:], mlpw[:, 64:128], start=False, stop=False)
        nc.tensor.matmul(f1p[:], ones[:], mlpb[0:1, 0:64], start=False, stop=True)
        r1 = const.tile([64, 64], F32)
        nc.scalar.activation(r1[:], f1p[:], AF.Relu)
        rsp = ppool.tile([64, 64], F32, tag='mlp')
        nc.tensor.matmul(rsp[:], ctx0[:], mlpw[:, 128:192], start=True, stop=False)
        nc.tensor.matmul(rsp[:], ctx1[:], mlpw[:, 192:256], start=False, stop=False)
        nc.tensor.matmul(rsp[:], ones[:], mlpb[0:1, 64:128], start=False, stop=True)
        u = const.tile([64, 64], F32)
        nc.vector.tensor_tensor(u[:], r1[:], rsp[:], op=ALU.add)
        uptp = ppool.tile([64, 64], F32, tag='mlp')
        nc.tensor.transpose(uptp[:], u[:], ident[0:64, 0:64])
        uT = const.tile([64, 64], F32)
        nc.scalar.copy(uT[:], uptp[:])
        f2p = ppool.tile([64, 64], F32, tag='mlp')
        nc.tensor.matmul(f2p[:], mlpw[0:64, 256:320], uT[:], start=True, stop=False)
        nc.tensor.matmul(f2p[:], mlpb[0:1, 128:192], ones[:], start=False, stop=True)
        r2 = const.tile([64, 64], F32)
        nc.scalar.activation(r2[:], f2p[:], AF.Relu)
        op = ppool.tile([64, 6], F32, tag='mlp')
        nc.tensor.matmul(op[:], r2[:], mlpw[0:64, 320:326], start=True, stop=False)
        nc.tensor.matmul(op[:], ones[:], mlpb[0:1, 192:198], start=False, stop=True)
        ofin = const.tile([64, 6], F32)
        nc.scalar.copy(ofin[:], op[:])
        nc.sync.dma_start(out_d[:], ofin[:])
    nc.compile()
    return nc


_CACHE = {}


def kernel(x, params):
    x = np.asarray(x, np.float32)
    shared = _prep_shared(params)
    if T not in _CACHE:
        _CACHE[T] = build(T)
    nc = _CACHE[T]
    in_maps = []
    for i in range(NCORES):
        m = dict(shared)
        m['xt'] = _prep_x(x[i * BL:(i + 1) * BL])
        in_maps.append(m)
    res = run_bass_kernel_spmd(nc, in_maps, list(range(NCORES)))
    outs = [np.asarray(res.results[i]['out'], np.float32) for i in range(NCORES)]
    return np.concatenate(outs, axis=0)


if __name__ == '__main__':
    build(64)
    print('build ok')
